# revision 24
# baseline (speedup 1.0000x reference)
"""BertSelfAttention with relative inference-path bias — Bass/Tile TRN2 kernel.

Shapes: B=2, S=128, H=12, DH=64, HID=768.  8 NeuronCores.

The reference materializes ip = inference_path @ Wip ([B,S,S,1536], 201MB)
and 5D ra/rb tensors; it reinterprets ra/rb via a RAW flat view
(`.reshape(B,H,S,S,DH)` on a [B,S,S,HID] array — torch .view semantics),
which scrambles cells: score pair w = (h,q') with w = h*128+q' draws its
bias row from raw cells t = 12*k_raw + c_idx of projection block
q_raw = w // 12, where (j, k') = divmod(t, 128), j = w - 12*q_raw.

Sharding: (b, q_raw) blocks, 32 per core; each core therefore covers score
pairs w in [384*c4, 384*(c4+1)) = 3 heads x all 128 queries of its batch,
and emits a PARTIAL mlp output summed on the host across the 4 cores of
each batch.

Per (b, q_raw): transpose IP tile on PE, project IP@[Wa|Wb] (fp32r,
1 cyc/row), round-trip the projection through DRAM to convert the raw
flat view into slab-ordered [k', (j,d)] tiles (slab j is a contiguous
32KB range of the flat projection), fold q via GPSIMD partition-broadcast
+ add, fold k via broadcast views on DVE, multiply + segmented-reduce to
scores. Softmax over k' (partition dim) via ones-matmul denominators on
PE and one batched ACT exp with fused mask-bias/scale.
"""
import math
import sys

import numpy as np

if '/opt/trn_rl_repo' not in sys.path:
    sys.path.insert(0, '/opt/trn_rl_repo')

H = 12
DH = 64
HID = 768
B = 2
S = 128
N_CORES = 8
QS = (B * S) // N_CORES   # 32 q_raw blocks per core
NH = 3                    # heads per core
HS = NH * DH              # 192
HSP = 256                 # padded head-slice width (fp32r needs N>=256)

_CACHE = {}


def _build_program(rt_bf16=True, debug=False):
    import concourse.bass as bass
    import concourse.bacc as bacc
    import concourse.mybir as mybir
    import concourse.tile as tile

    f32 = mybir.dt.float32
    f32r = mybir.dt.float32r
    bf16 = mybir.dt.bfloat16
    rt = bf16 if rt_bf16 else f32   # round-trip dtype for scrambled tensors
    AX = mybir.AxisListType
    OP = mybir.AluOpType
    ACTF = mybir.ActivationFunctionType

    nc = bacc.Bacc("TRN2", target_bir_lowering=False, debug=False,
                   num_devices=N_CORES)

    # ---- DRAM I/O (per-core shapes) ----
    ip_d = nc.dram_tensor("ip", [QS, S, HID], f32, kind="ExternalInput").ap()
    hst_d = nc.dram_tensor("hst", [HID, S], f32r, kind="ExternalInput").ap()
    mask_d = nc.dram_tensor("mask", [S, 1], f32, kind="ExternalInput").ap()
    spant_d = nc.dram_tensor("spant", [S, S], f32r, kind="ExternalInput").ap()
    wq_d = nc.dram_tensor("wq", [HID, HSP], f32r, kind="ExternalInput").ap()
    wk_d = nc.dram_tensor("wk", [HID, HSP], f32r, kind="ExternalInput").ap()
    wv_d = nc.dram_tensor("wv", [HID, HSP], f32r, kind="ExternalInput").ap()
    wpv_d = nc.dram_tensor("wpv", [HID, DH], f32r, kind="ExternalInput").ap()
    wip_d = nc.dram_tensor("wip", [HID, 2 * HID], bf16, kind="ExternalInput").ap()
    wma_d = nc.dram_tensor("wmlpa", [128, HID], f32r, kind="ExternalInput").ap()
    wmb_d = nc.dram_tensor("wmlpb", [128, HID], f32r, kind="ExternalInput").ap()
    bq_d = nc.dram_tensor("bq", [1, HSP], f32r, kind="ExternalInput").ap()
    bk_d = nc.dram_tensor("bk", [1, HSP], f32r, kind="ExternalInput").ap()
    bv_d = nc.dram_tensor("bv", [1, HSP], f32r, kind="ExternalInput").ap()
    bpv_d = nc.dram_tensor("bpv", [1, DH], f32r, kind="ExternalInput").ap()
    bmlp_d = nc.dram_tensor("bmlp", [1, HID], f32r, kind="ExternalInput").ap()
    ones_d = nc.dram_tensor("ones", [S, 1], f32, kind="ExternalInput").ap()
    onesr_d = nc.dram_tensor("onesr", [1, S], f32r, kind="ExternalInput").ap()
    ident_d = nc.dram_tensor("ident", [S, S], f32, kind="ExternalInput").ap()
    out_d = nc.dram_tensor("out", [S, HID], f32, kind="ExternalOutput").ap()
    if debug:
        dbg = {nm: nc.dram_tensor(nm, shp, f32, kind="ExternalOutput").ap()
               for nm, shp in (("d_score", [S, H * QS]), ("d_qe2", [S, HID]),
                               ("d_ket", [S, HID]), ("d_qt2", [QS, HID]),
                               ("d_kh", [S, HS]), ("d_expn", [S, H * QS]),
                               ("d_ctxt", [S, 2 * S]))}

    with tile.TileContext(nc) as tc:
        with (
            tc.tile_pool(name="wpool", bufs=1) as wpool,
            tc.tile_pool(name="cpool", bufs=1) as cpool,
            tc.tile_pool(name="iopool", bufs=5) as iopool,
            tc.tile_pool(name="mid", bufs=4) as mid,
            tc.tile_pool(name="ppool", bufs=2, space=bass.MemorySpace.PSUM) as ppool,
            tc.tile_pool(name="tpool", bufs=1, space=bass.MemorySpace.PSUM) as tpool,
            tc.tile_pool(name="dpool", bufs=5, space=bass.MemorySpace.DRAM) as dpool,
            tc.tile_pool(name="dpool1", bufs=1, space=bass.MemorySpace.DRAM) as dpool1,
        ):
            # ---- constants / weights to SBUF ----
            ones = cpool.tile([S, 1], f32, tag="ones")
            nc.sync.dma_start(ones[:], ones_d)
            onesr = cpool.tile([1, S], f32r, tag="onesr")
            nc.sync.dma_start(onesr[:], onesr_d)
            ident = cpool.tile([S, S], f32, tag="ident")
            nc.sync.dma_start(ident[:], ident_d)
            mask = cpool.tile([S, 1], f32, tag="mask")
            nc.sync.dma_start(mask[:], mask_d)
            spant = cpool.tile([S, S], f32r, tag="spant")
            nc.sync.dma_start(spant[:], spant_d)
            hst = cpool.tile([S, HID], f32r, tag="hst")
            for i in range(6):
                nc.sync.dma_start(hst[:, i * 128:(i + 1) * 128],
                                  hst_d[i * 128:(i + 1) * 128, :])
            biases = {}
            for nm, d, w in (("bq", bq_d, HSP), ("bk", bk_d, HSP),
                             ("bv", bv_d, HSP), ("bpv", bpv_d, DH),
                             ("bmlp", bmlp_d, HID)):
                t = cpool.tile([1, w], f32r, tag=nm)
                nc.sync.dma_start(t[:], d)
                biases[nm] = t

            wip = []
            for i in range(6):
                t = wpool.tile([128, 2 * HID], bf16, tag=f"wip{i}")
                nc.sync.dma_start(t[:], wip_d[i * 128:(i + 1) * 128, :])
                wip.append(t)
            wqkv = {}
            for nm, d in (("wq", wq_d), ("wk", wk_d), ("wv", wv_d)):
                ch = []
                for i in range(6):
                    t = wpool.tile([128, HSP], f32r, tag=f"{nm}{i}")
                    nc.sync.dma_start(t[:], d[i * 128:(i + 1) * 128, :])
                    ch.append(t)
                wqkv[nm] = ch
            wpv = []
            for i in range(6):
                t = wpool.tile([128, DH], f32r, tag=f"wpv{i}")
                nc.sync.dma_start(t[:], wpv_d[i * 128:(i + 1) * 128, :])
                wpv.append(t)
            wma = wpool.tile([128, HID], f32r, tag="wma")
            nc.sync.dma_start(wma[:], wma_d)
            wmb = wpool.tile([128, HID], f32r, tag="wmb")
            nc.sync.dma_start(wmb[:], wmb_d)

            # ---- phase Q: 3-head q/k/v + pv projections ----
            def head_proj(wch, bias):
                ps = tpool.tile([S, HID], f32, tag="tp")
                for i in range(6):
                    nc.tensor.matmul(ps[:, 0:HSP],
                                     hst[:, i * 128:(i + 1) * 128],
                                     wch[i][:], start=(i == 0), stop=False)
                nc.tensor.matmul(ps[:, 0:HSP], onesr[:, 0:128], bias[:],
                                 start=False, stop=True)
                return ps

            ps = head_proj(wqkv["wk"], biases["bk"])
            kh_sb = cpool.tile([S, HS], rt, tag="kh_sb")
            nc.scalar.copy(kh_sb[:], ps[:, 0:HS])
            ps = head_proj(wqkv["wv"], biases["bv"])
            vh_sb = cpool.tile([S, HS], f32, tag="vh_sb")
            nc.vector.tensor_copy(vh_sb[:], ps[:, 0:HS])
            ps = head_proj(wqkv["wq"], biases["bq"])
            qh_sb = cpool.tile([S, HS], rt, tag="qh_sb")
            nc.scalar.copy(qh_sb[:], ps[:, 0:HS])
            # qt2[qi, j*64+d] = Qh[q', hl*64+d], (hl,q') = divmod(12*qi+j, 128)
            qt2_dram = dpool1.tile([QS, HID], rt, tag="qt2")
            nc.sync.dma_start(
                qt2_dram[:].flatten().rearrange("(h q d) -> q h d",
                                                h=NH, q=S, d=DH),
                qh_sb[:].rearrange("q (h d) -> q h d", h=NH, d=DH))

            ps = tpool.tile([S, HID], f32, tag="tp")
            for i in range(6):
                nc.tensor.matmul(ps[:, 0:DH], hst[:, i * 128:(i + 1) * 128],
                                 wpv[i][:], start=(i == 0), stop=False)
            nc.tensor.matmul(ps[:, 0:DH], onesr[:, 0:128], biases["bpv"][:],
                             start=False, stop=True)
            pv_sb = cpool.tile([S, DH], f32, tag="pv_sb")
            nc.scalar.copy(pv_sb[:], ps[:, 0:DH])

            # ---- main loop over this core's 32 q_raw blocks ----
            # Software-pipelined: stage A (load/transpose/project/write) at
            # qi; stage B (slab reads + folds + scores) lagged by 2 so the
            # DMA round-trip latency never stalls the sequencers.
            score_all = cpool.tile([S, H * QS], f32, tag="score_all")
            stash = {}

            def stage_a(qi):
                ip_sb = iopool.tile([S, HID], f32, tag="ip")
                nc.sync.dma_start(ip_sb[:], ip_d[qi])
                pr = ppool.tile([S, 2 * HID], f32, tag="proj")
                for i in range(6):
                    nc.tensor.transpose(pr[:, i * 128:(i + 1) * 128],
                                        ip_sb[:, i * 128:(i + 1) * 128],
                                        ident[:])
                ipt = mid.tile([S, HID], bf16, tag="ipt")
                nc.scalar.copy(ipt[:], pr[:, 0:HID])
                for i in range(6):
                    lhs = ipt[:, i * 128:(i + 1) * 128]
                    st = (i == 0)
                    sp = (i == 5)
                    nc.tensor.matmul(pr[:, 0:512], lhs, wip[i][:, 0:512],
                                     start=st, stop=sp)
                    nc.tensor.matmul(pr[:, 512:1024], lhs, wip[i][:, 512:1024],
                                     start=st, stop=sp)
                    nc.tensor.matmul(pr[:, 1024:1536], lhs, wip[i][:, 1024:1536],
                                     start=st, stop=sp)
                proj_sb = mid.tile([S, 2 * HID], rt, tag="proj_sb")
                nc.vector.tensor_copy(proj_sb[:], pr[:])
                pda = dpool.tile([S, HID], rt, tag="pda")
                nc.gpsimd.dma_start(pda[:], proj_sb[:, 0:HID])
                pdb = dpool.tile([S, HID], rt, tag="pdb")
                nc.gpsimd.dma_start(pdb[:], proj_sb[:, HID:2 * HID])
                qcat = iopool.tile([1, HID], rt, tag="qcat")
                nc.gpsimd.dma_start(qcat[:], qt2_dram[qi:qi + 1, :])
                stash[qi] = (pda, pdb, qcat)

            def stage_b(qi):
                pda, pdb, qcat = stash.pop(qi)
                qe_t = iopool.tile([S, HID], rt, tag="qe_t")
                nc.sync.dma_start(
                    qe_t[:].rearrange("k (j d) -> k j d", j=H, d=DH),
                    pda[:].flatten().rearrange("(j k d) -> k j d",
                                               j=H, k=S, d=DH))
                ke_t = iopool.tile([S, HID], rt, tag="ke_t")
                nc.sync.dma_start(
                    ke_t[:].rearrange("k (j d) -> k j d", j=H, d=DH),
                    pdb[:].flatten().rearrange("(j k d) -> k j d",
                                               j=H, k=S, d=DH))
                qbc = mid.tile([S, HID], rt, tag="qbc")
                nc.gpsimd.partition_broadcast(qbc[:], qcat[0:1, :])
                qe2 = mid.tile([S, HID], rt, tag="qe2")
                nc.gpsimd.tensor_add(qe2[:], qe_t[:], qbc[:])
                hl0 = (12 * qi) // 128
                jsplit = min(12, 128 * (hl0 + 1) - 12 * qi)
                ranges = (((0, jsplit, hl0),) if jsplit >= 12 else
                          ((0, jsplit, hl0), (jsplit, 12, hl0 + 1)))
                for (jlo, jhi, hl) in ranges:
                    cnt = jhi - jlo
                    nc.vector.tensor_add(
                        ke_t[:, jlo * DH:jhi * DH].rearrange(
                            "k (j d) -> k j d", j=cnt),
                        ke_t[:, jlo * DH:jhi * DH].rearrange(
                            "k (j d) -> k j d", j=cnt),
                        kh_sb[:, hl * DH:(hl + 1) * DH].rearrange(
                            "k (o d) -> k o d", o=1).broadcast_to([S, cnt, DH]))
                prod = mid.tile([S, HID], rt, tag="prod")
                nc.vector.tensor_mul(prod[:], qe2[:], ke_t[:])
                nc.vector.tensor_reduce(
                    score_all[:, 12 * qi:12 * qi + 12],
                    prod[:].rearrange("p (j d) -> p j d", j=H),
                    axis=AX.X, op=OP.add)

            LAG = 3
            for qi in range(QS):
                stage_a(qi)
                if qi >= LAG:
                    stage_b(qi - LAG)
            for qi in range(QS - LAG, QS):
                stage_b(qi)

            # ---- batched softmax over k' (partitions) ----
            expall = cpool.tile([S, H * QS], f32, tag="expall")
            nc.scalar.activation(expall[:], score_all[:], ACTF.Exp,
                                 bias=mask[:, 0:1], scale=1.0 / math.sqrt(DH))
            pden = tpool.tile([S, HID], f32, tag="tp")
            nc.tensor.matmul(pden[0:1, 0:H * QS], ones[:, 0:1], expall[:],
                             start=True, stop=True)
            recip = cpool.tile([1, H * QS], f32, tag="recip")
            nc.vector.reciprocal(recip[:], pden[0:1, 0:H * QS])
            pdenb = tpool.tile([S, HID], f32, tag="tp")
            nc.tensor.matmul(pdenb[:, 0:H * QS], onesr[:, 0:128].bitcast(f32),
                             recip[:], start=True, stop=True)
            expn = cpool.tile([S, H * QS], f32, tag="expn")
            nc.vector.tensor_mul(expn[:], expall[:], pdenb[:, 0:H * QS])

            if debug:
                nc.sync.dma_start(dbg["d_expn"], expn[:])
            # ---- ctx^T blocks + parse block: [128, 256] ----
            # chunk A (cols 0:128):  partitions 0:64 = hl0, 64:128 = hl1
            # chunk B (cols 128:256): partitions 0:64 = hl2, 64:128 = parse
            pctx = tpool.tile([S, HID], f32, tag="tp")
            for hl in range(NH):
                po = (hl % 2) * DH
                co = (hl // 2) * S
                nc.tensor.matmul(pctx[po:po + DH, co:co + S],
                                 vh_sb[:, hl * DH:(hl + 1) * DH],
                                 expn[:, hl * S:(hl + 1) * S],
                                 start=True, stop=True)
            nc.tensor.matmul(pctx[DH:2 * DH, S:2 * S], pv_sb[:],
                             spant[:].bitcast(f32), start=True, stop=True)
            ctxt = cpool.tile([S, 2 * S], f32r, tag="ctxt")
            nc.scalar.copy(ctxt[:], pctx[:, 0:2 * S])

            # ---- partial mlp: out[q', o] = ctxA.T@wmlpA + ctxB.T@wmlpB ----
            pout = ppool.tile([S, 2 * HID], f32, tag="proj")
            nc.tensor.matmul(pout[:, 0:512], ctxt[:, 0:S], wma[:, 0:512],
                             start=True, stop=False)
            nc.tensor.matmul(pout[:, 512:768], ctxt[:, 0:S], wma[:, 512:768],
                             start=True, stop=False)
            nc.tensor.matmul(pout[:, 0:512], ctxt[:, S:2 * S], wmb[:, 0:512],
                             start=False, stop=False)
            nc.tensor.matmul(pout[:, 512:768], ctxt[:, S:2 * S], wmb[:, 512:768],
                             start=False, stop=False)
            nc.tensor.matmul(pout[:, 0:512], onesr[:, 0:128],
                             biases["bmlp"][:, 0:512], start=False, stop=False)
            nc.tensor.matmul(pout[:, 512:768], onesr[:, 0:128],
                             biases["bmlp"][:, 512:768], start=False, stop=True)
            if debug:
                dct = cpool.tile([S, 2 * S], f32, tag="dct")
                nc.vector.tensor_copy(dct[:], ctxt[:].bitcast(f32))
                nc.sync.dma_start(dbg["d_ctxt"], dct[:])
            out_sb = cpool.tile([S, HID], f32, tag="out_sb")
            nc.vector.tensor_copy(out_sb[:], pout[:, 0:768])
            nc.sync.dma_start(out_d, out_sb[:])

    nc.compile()
    return nc


def _make_in_maps(inputs):
    a = {k: np.ascontiguousarray(np.asarray(v, dtype=np.float32))
         for k, v in inputs.items()}
    zpad = np.zeros((HID, HSP - HS), np.float32)
    import ml_dtypes
    wip_bf = a["Wip"].astype(ml_dtypes.bfloat16)

    in_maps = []
    for c in range(N_CORES):
        b = (c * QS) // S
        q0 = (c * QS) % S
        c4 = c % 4
        h0 = NH * c4
        wq_s = np.concatenate([a["Wq"][:, h0 * DH:(h0 + NH) * DH], zpad], 1)
        wk_s = np.concatenate([a["Wk"][:, h0 * DH:(h0 + NH) * DH], zpad], 1)
        wv_s = np.concatenate([a["Wv"][:, h0 * DH:(h0 + NH) * DH], zpad], 1)
        bpad = np.zeros(HSP - HS, np.float32)
        bq_s = np.concatenate([a["bq"][h0 * DH:(h0 + NH) * DH], bpad])
        bk_s = np.concatenate([a["bk"][h0 * DH:(h0 + NH) * DH], bpad])
        bv_s = np.concatenate([a["bv"][h0 * DH:(h0 + NH) * DH], bpad])
        spant = (a["span_mask"][b, 0].T if c4 == 0
                 else np.zeros((S, S), np.float32))
        bmlp = a["bmlp"] if c4 == 0 else np.zeros(HID, np.float32)
        wmlpa = a["Wmlp"][HS * c4:HS * c4 + 128]
        wmlpb = np.concatenate([a["Wmlp"][HS * c4 + 128:HS * (c4 + 1)],
                                a["Wmlp"][HID:HID + DH]], 0)
        in_maps.append({
            "ip": a["inference_path"][b, q0:q0 + QS],
            "hst": np.ascontiguousarray(a["hidden_states"][b].T),
            "mask": a["attention_mask"][b, 0, 0].reshape(S, 1),
            "spant": np.ascontiguousarray(spant),
            "wq": wq_s, "wk": wk_s, "wv": wv_s,
            "wpv": a["Wpv"], "wip": wip_bf,
            "wmlpa": np.ascontiguousarray(wmlpa),
            "wmlpb": np.ascontiguousarray(wmlpb),
            "bq": bq_s.reshape(1, HSP), "bk": bk_s.reshape(1, HSP),
            "bv": bv_s.reshape(1, HSP),
            "bpv": a["bpv"].reshape(1, DH),
            "bmlp": bmlp.reshape(1, HID),
            "ones": np.ones((S, 1), np.float32),
            "onesr": np.ones((1, S), np.float32),
            "ident": np.eye(S, dtype=np.float32),
        })
    return in_maps


def kernel(**inputs):
    if "nc" not in _CACHE:
        _CACHE["nc"] = _build_program()
    nc = _CACHE["nc"]
    in_maps = _make_in_maps(inputs)

    from concourse import bass_utils
    res = bass_utils.run_bass_kernel_spmd(nc, in_maps, core_ids=list(range(N_CORES)))
    out = np.zeros((B, S, HID), np.float32)
    for c in range(N_CORES):
        b = (c * QS) // S
        out[b] += res.results[c]["out"]
    return out
